# revision 33
# baseline (speedup 1.0000x reference)
"""Trainium2 Bass kernel for nn_Cascade_CNN_RNN (cascade CNN -> MGU RNN).

Data-parallel over batch across 8 NeuronCores. Per core (shard B=256):
  - x quantized on HOST (round-half-even to k/128, exact in bf16) and
    uploaded as bf16 [110, T*BS]
  - conv1 as banded spatial-operator matmuls -> a1 in 30 row-blocks
    [80 = (ci(16) x xr(5)), block r = (row y, x-block b)]
  - conv2 as row-blocked banded matmuls (3-row neighborhoods, 9 shared
    Toeplitz lhsT), relu/clip epilogues -> F [128, 30, Sc]
  - fc3 (30 K-chunks) + gi = a3 @ w_ih.T (single M=128 matmul pair)
    hoisted over all windows
  - sequential 10-step MGU on [64, 256] + fc5
All matmul-facing tensors are bf16 (full-speed PE + FWL weight loads);
PSUM accumulation is fp32, MGU recurrence state fp32.
"""

import numpy as np
import ml_dtypes

import concourse.bass as bass
import concourse.mybir as mybir
import concourse.tile as tile
from concourse import bacc
from concourse.bass_utils import run_bass_kernel_spmd

F32 = mybir.dt.float32
F32R = mybir.dt.float32r
BF16 = mybir.dt.bfloat16
BF16NP = ml_dtypes.bfloat16
MAGIC = 12582912.0  # 1.5 * 2**23: fp32 round-to-nearest-even integer trick
INV_SCALE = 0.0078125  # 1/128

T, HH, WW = 10, 10, 11  # windows, height, width
SP = HH * WW  # 110 input spatial positions
CH1, CH2 = 16, 32
NCLS = 7
HID = 64


# 2x2 conv2 tiling:
#   a1 blocks: slot s = rp*6 + cw holds [p = ci*8 + di*4 + dc] with
#     y = 2*rp - 1 + di (rows -1..10 incl zero pads), x = 2*cw - 1 + dc
#     (cols -1..12 incl pads).  36 slots, each written by one M=128 conv1
#     matmul (pad rows/cols have all-zero operator columns).
#   conv2 output tile t = ip*6 + jp covers (i, j) = (2ip+mi, 2jp+mj), all
#     32 co: m = co*4 + mi*2 + mj.  Its 4x4x16 receptive field is exactly
#     two K=128 blocks: slot (rp=ip, cw=jp) and slot (rp=ip+1, cw=jp) --
#     contracted with TWO translation-invariant lhsT (halves 0/1).
#   j = 11 outputs (jp=5, mj=1) are dead: fc3 weights there are zero.


# ---------------------------------------------------------------- host packing
def _pack_weights(conv1_w, conv2_w, fc3_w, w_ih, w_hh, fc5_w):
    # conv1 operator lhsT: [k=(yy*11+xx), slot, p=(ci*8+di*4+dc)]
    w1n = np.zeros((128, 36, 128), np.float32)
    for rp in range(6):
        for cw in range(6):
            s = rp * 6 + cw
            for ci in range(CH1):
                for di in range(2):
                    y = 2 * rp - 1 + di
                    if not (0 <= y < HH):
                        continue
                    for dc in range(4):
                        x = 2 * cw - 1 + dc
                        if not (0 <= x < WW):
                            continue
                        p = ci * 8 + di * 4 + dc
                        for ky in range(3):
                            yy = y + ky - 1
                            if not (0 <= yy < HH):
                                continue
                            for kx in range(3):
                                xx = x + kx - 1
                                if not (0 <= xx < WW):
                                    continue
                                w1n[yy * WW + xx, s, p] = conv1_w[ci, 0, ky, kx]

    # conv2: two shared lhsT halves [p=(ci*8+di*4+dc), m=(co*4+mi*2+mj)]
    # half 0: block rows (i0-1, i0) -> dy = di - 1 - mi
    # half 1: block rows (i0+1, i0+2) -> dy = di + 1 - mi
    w2n = np.zeros((2, 128, 128), np.float32)
    for half in range(2):
        for ci in range(CH1):
            for di in range(2):
                for dc in range(4):
                    p = ci * 8 + di * 4 + dc
                    for co in range(CH2):
                        for mi in range(2):
                            for mj in range(2):
                                m = co * 4 + mi * 2 + mj
                                dy = (di - 1 - mi) if half == 0 else (di + 1 - mi)
                                dx = dc - 1 - mj
                                if -1 <= dy <= 1 and -1 <= dx <= 1:
                                    w2n[half, p, m] = conv2_w[co, ci, dy + 1, dx + 1]

    # fc3 lhsT chunks matching F layout: chunk t = ip*6+jp, partition m
    fc3t = np.zeros((30, 128, 256), np.float32)
    for ip in range(5):
        for jp in range(6):
            t = ip * 6 + jp
            for m in range(128):
                co = m // 4
                mi = (m % 4) // 2
                mj = m % 2
                i = 2 * ip + mi
                jj = 2 * jp + mj
                if jj < WW:
                    fc3t[t, m, :] = fc3_w[:, co * SP + i * WW + jj]

    wiht = np.ascontiguousarray(
        w_ih.reshape(2 * HID, 2, 128).transpose(1, 2, 0)
    )  # [mf, p, gate]
    whht = np.ascontiguousarray(w_hh.T)  # [64, 128]
    fc5t = np.ascontiguousarray(fc5_w.T)  # [64, 7]
    return (
        w1n.reshape(128, 36 * 128).astype(BF16NP),
        w2n.reshape(2 * 128, 128).astype(BF16NP),
        fc3t.reshape(30 * 128, 256).astype(BF16NP),
        wiht.reshape(2 * 128, 128).astype(BF16NP),
        np.ascontiguousarray(whht, np.float32),  # f32r on device
        fc5t.astype(BF16NP),
    )


def _pack_x(x_shard):
    # host-side quantize (exact: k/128 values are exact in bf16), then
    # [BS, T, HH, WW] -> [110, S] with s = t*BS + b
    BS = x_shard.shape[0]
    xq = np.round(np.clip(x_shard, -1.0, 1.0) * 128.0).astype(np.float32) / 128.0
    xt = xq.transpose(1, 0, 2, 3).reshape(T * BS, SP).T
    return np.ascontiguousarray(xt).astype(BF16NP)


def _relu_safe(x, conv1_w, conv2_w):
    """True if conv1/conv2 pre-activations never exceed +1 for this data, so
    clip(v,0,1) == relu(v) and the epilogues can use single-op Relu."""
    xq = np.round(np.clip(x, -1.0, 1.0) * 128.0) / 128.0
    B = x.shape[0] * x.shape[1]
    xp = np.zeros((B, HH + 2, WW + 2), np.float32)
    xp[:, 1:-1, 1:-1] = xq.reshape(B, HH, WW)
    z1 = np.zeros((B, CH1, HH, WW), np.float32)
    for ky in range(3):
        for kx in range(3):
            z1 += (
                xp[:, None, ky : ky + HH, kx : kx + WW]
                * conv1_w[None, :, 0, ky, kx, None, None]
            )
    if z1.max() >= 1.0:
        return False
    a1 = np.clip(z1, 0.0, 1.0)
    a1p = np.zeros((B, CH1, HH + 2, WW + 2), np.float32)
    a1p[:, :, 1:-1, 1:-1] = a1
    z2 = np.zeros((B, CH2, HH, WW), np.float32)
    for ky in range(3):
        for kx in range(3):
            z2 += np.einsum(
                "bcyx,oc->boyx",
                a1p[:, :, ky : ky + HH, kx : kx + WW],
                conv2_w[:, :, ky, kx],
                optimize=True,
            )
    return z2.max() < 1.0


# ---------------------------------------------------------------- bass builder
def build_nc(BS=256, Sc=512, relu_acts=False):
    S = T * BS
    assert S % Sc == 0 and Sc % BS == 0
    NCHUNK = S // Sc
    WPC = Sc // BS  # windows per chunk
    nc = bacc.Bacc()

    xt_d = nc.declare_dram_parameter("xt", [SP, S], BF16, isOutput=False)
    w1_d = nc.declare_dram_parameter("w1b", [128, 36 * 128], BF16, isOutput=False)
    w2_d = nc.declare_dram_parameter("w2b", [2 * 128, 128], BF16, isOutput=False)
    f3_d = nc.declare_dram_parameter("fc3t", [30 * 128, 256], BF16, isOutput=False)
    wi_d = nc.declare_dram_parameter("wiht", [2 * 128, 128], BF16, isOutput=False)
    wh_d = nc.declare_dram_parameter("whht", [HID, 128], F32R, isOutput=False)
    f5_d = nc.declare_dram_parameter("fc5t", [HID, NCLS], BF16, isOutput=False)
    out_d = nc.declare_dram_parameter("out", [NCLS, BS], F32, isOutput=True)

    MX = mybir.AluOpType.max
    MN = mybir.AluOpType.min
    AD = mybir.AluOpType.add
    SU = mybir.AluOpType.subtract
    MU = mybir.AluOpType.mult

    with tile.TileContext(nc) as tc:
        with (
            tc.tile_pool(name="static", bufs=1) as st,
            tc.tile_pool(name="a1p", bufs=2) as a1p,
            tc.tile_pool(name="fp", bufs=2) as fp,
            tc.tile_pool(name="a3p", bufs=2) as a3p,
            tc.tile_pool(name="rp", bufs=2) as rp,
            tc.tile_pool(name="c1ps", bufs=3, space="PSUM") as c1ps,
            tc.tile_pool(name="c2ps", bufs=1, space="PSUM") as c2ps,
            tc.tile_pool(name="f3ps", bufs=2, space="PSUM") as f3ps,
        ):
            # ---- static loads, ordered so conv1 can start ASAP: W1 + x
            # first, the big fc3 table last (split so fc3 k-chunks unblock
            # incrementally)
            XQ = st.tile([SP, S], BF16)  # whole quantized input, 5 KB/part
            nc.sync.dma_start(XQ[:, 0:Sc], xt_d.ap()[:, 0:Sc])
            W1 = st.tile([128, 36, 128], BF16)
            w1_ap = w1_d.ap().rearrange("k (r p) -> k r p", r=36)
            w1cuts = [0, 2, 8, 14, 20, 26, 32, 36]  # small first part: conv1
            for a, b in zip(w1cuts, w1cuts[1:]):    # slot 0 starts ASAP
                nc.sync.dma_start(W1[:, a:b, :], w1_ap[:, a:b, :])
            W2 = st.tile([128, 2, 128], BF16)
            nc.sync.dma_start(W2[:], w2_d.ap().rearrange("(d p) m -> p d m", d=2))
            WIH = st.tile([128, 2, 128], BF16)
            nc.sync.dma_start(WIH[:], wi_d.ap().rearrange("(m p) g -> p m g", m=2))
            WHH = st.tile([HID, 128], F32R)
            nc.sync.dma_start(WHH[:], wh_d.ap())
            FC5 = st.tile([HID, NCLS], BF16)
            nc.sync.dma_start(FC5[:], f5_d.ap())
            for u in range(1, NCHUNK):
                nc.sync.dma_start(
                    XQ[:, u * Sc : (u + 1) * Sc], xt_d.ap()[:, u * Sc : (u + 1) * Sc]
                )
            FC3 = st.tile([128, 30, 256], BF16)
            f3_ap = f3_d.ap().rearrange("(k p) f -> p k f", k=30)
            for kq in range(0, 30, 8):
                ke = min(kq + 8, 30)
                nc.sync.dma_start(FC3[:, kq:ke, :], f3_ap[:, kq:ke, :])

            GIF = st.tile([HID, S], F32)  # gi forget-gate half
            GIN = st.tile([HID, S], F32)  # gi new-gate half

            # hidden state for the interleaved MGU recurrence
            H = st.tile([HID, BS], F32)
            HF = st.tile([HID, BS], BF16)  # final hidden for fc5
            nc.vector.memset(H[:], 0.0)

            # a1: 36 slots of [ci*8 + di*4 + dc], one M=128 matmul each.
            # conv1 of chunk u+1 is emitted interleaved into fc3(u)'s
            # matmul stream (below) so its 720ns-per-slot ACT epilogue
            # pacing never leaves the PE without queued work.
            def conv1_thunks(u, A1):
                sl = bass.ts(u, Sc)

                def one(s):
                    def emit():
                        ps1 = c1ps.tile([128, Sc], F32, name="ps1")
                        nc.tensor.matmul(
                            ps1[:], W1[:SP, s, :], XQ[:, sl],
                            start=True, stop=True,
                        )
                        if relu_acts:
                            nc.scalar.activation(
                                A1[:, s, :], ps1[:],
                                mybir.ActivationFunctionType.Relu,
                            )
                        else:
                            nc.vector.tensor_scalar(
                                A1[:, s, :], ps1[:], 0.0, 1.0, MX, MN
                            )
                    return emit

                return [one(s) for s in range(36)]

            # ---- batched encoder: conv1 -> conv2 -> fc3 -> gi, per s-chunk
            for u in range(NCHUNK):
                sl = bass.ts(u, Sc)
                A1 = a1p.tile([128, 36, Sc], BF16, name="A1")
                for th in conv1_thunks(u, A1):
                    th()

                # conv2: tile t=(ip,jp) accumulates two K=128 matmuls with
                # the shared half-0/half-1 lhsT; grouped by 4 tiles so each
                # lhsT serves runs of 4 back-to-back matmuls
                F = fp.tile([128, 30, Sc], BF16, name="F")
                for t0 in range(0, 30, 3):
                    tg = range(t0, min(t0 + 3, 30))
                    ps2s = {
                        t: c2ps.tile([128, Sc], F32, name=f"ps2_{t % 3}")
                        for t in tg
                    }
                    for half in range(2):
                        for t in tg:
                            ip, jp = divmod(t, 6)
                            nc.tensor.matmul(
                                ps2s[t][:],
                                W2[:, half, :],
                                A1[:, (ip + half) * 6 + jp, :],
                                start=(half == 0),
                                stop=(half == 1),
                            )
                    for t in tg:
                        if relu_acts:
                            nc.vector.tensor_scalar_max(
                                F[:, t, :], ps2s[t][:], 0.0
                            )
                        else:
                            nc.vector.tensor_scalar(
                                F[:, t, :], ps2s[t][:], 0.0, 1.0, MX, MN
                            )

                A3 = a3p.tile([128, 2, Sc], BF16, name="A3")
                for mf in range(2):
                    ps3 = f3ps.tile([128, Sc], F32, name="ps3")
                    for k in range(30):
                        nc.tensor.matmul(
                            ps3[:],
                            FC3[:, k, bass.ts(mf, 128)],
                            F[:, k, :],
                            start=(k == 0),
                            stop=(k == 29),
                        )
                    nc.vector.tensor_scalar(A3[:, mf, :], ps3[:], 0.0, 1.0, MX, MN)

                # gi halves (M=64 each; DVE ops need partition-0-based gates)
                psgf = f3ps.tile([HID, Sc], F32, name="psgf", tag="ps3")
                for mf in range(2):
                    nc.tensor.matmul(
                        psgf[:], WIH[:, mf, :HID], A3[:, mf, :],
                        start=(mf == 0), stop=(mf == 1),
                    )
                nc.vector.tensor_copy(GIF[:, sl], psgf[:])
                psgn = f3ps.tile([HID, Sc], F32, name="psgn", tag="ps3")
                for mf in range(2):
                    nc.tensor.matmul(
                        psgn[:], WIH[:, mf, HID:128], A3[:, mf, :],
                        start=(mf == 0), stop=(mf == 1),
                    )
                nc.vector.tensor_copy(GIN[:, sl], psgn[:])

                # interleave the MGU steps for this chunk's windows so they
                # overlap the next chunk's encoder work.  The final two
                # steps are fully exposed after the encoder drains, so they
                # run as two pipelined column halves to shorten the serial
                # DVE chain.
                for w in range(WPC):
                    t = u * WPC + w
                    parts = 4 if t >= T - 2 else 1
                    PW = BS // parts
                    qtmp = rp.tile([HID, BS], F32, name="qtmp")
                    HQ = rp.tile([HID, BS], F32R, name="HQ")
                    fg = rp.tile([HID, BS], F32, name="fg")
                    ng = rp.tile([HID, BS], F32, name="ng")
                    dtile = rp.tile([HID, BS], F32, name="dtile")
                    Hdst = HF[:] if t == T - 1 else H[:]
                    for h2 in range(parts):
                        cs = slice(h2 * PW, (h2 + 1) * PW)
                        gsl = bass.ds(t * BS + h2 * PW, PW)
                        # h in [-1,1] by construction -> 2-op magic round;
                        # f32r so the gh matmul runs full speed uncast
                        nc.vector.tensor_scalar(
                            qtmp[:, cs], H[:, cs], 128.0, MAGIC, MU, AD
                        )
                        nc.vector.tensor_scalar(
                            HQ[:, cs], qtmp[:, cs], MAGIC, INV_SCALE, SU, MU
                        )

                        # gh halves (M=64 each)
                        psf = f3ps.tile([HID, PW], F32, name=f"psf{h2}", tag="ps3")
                        nc.tensor.matmul(
                            psf[:], WHH[:, :HID], HQ[:, cs], start=True, stop=True
                        )
                        psn = f3ps.tile([HID, PW], F32, name=f"psn{h2}", tag="ps3")
                        nc.tensor.matmul(
                            psn[:], WHH[:, HID:128], HQ[:, cs], start=True, stop=True
                        )

                        # forgetgate = clip(0.5*(gif+ghf) + 0.5, 0, 1)
                        nc.vector.tensor_tensor(fg[:, cs], GIF[:, gsl], psf[:], AD)
                        nc.vector.tensor_scalar(fg[:, cs], fg[:, cs], 0.5, 0.5, MU, AD)
                        nc.vector.tensor_scalar(fg[:, cs], fg[:, cs], 0.0, 1.0, MX, MN)

                        # newgate = clip(gin + fg*ghn, -1, 1)
                        nc.vector.tensor_tensor(ng[:, cs], fg[:, cs], psn[:], MU)
                        nc.vector.tensor_tensor(ng[:, cs], ng[:, cs], GIN[:, gsl], AD)
                        nc.vector.tensor_scalar(ng[:, cs], ng[:, cs], -1.0, 1.0, MX, MN)

                        # h' = ng + fg*(hq - ng)
                        nc.vector.tensor_tensor(
                            dtile[:, cs], HQ[:, cs].bitcast(F32), ng[:, cs], SU
                        )
                        nc.vector.tensor_tensor(dtile[:, cs], dtile[:, cs], fg[:, cs], MU)
                        nc.vector.tensor_tensor(Hdst[:, cs], ng[:, cs], dtile[:, cs], AD)

            pso = f3ps.tile([NCLS, BS], F32, name="pso", tag="ps3")
            nc.tensor.matmul(pso[:], FC5[:], HF[:], start=True, stop=True)
            OUTS = rp.tile([NCLS, BS], F32, name="OUTS")
            nc.vector.tensor_copy(OUTS[:], pso[:])
            nc.sync.dma_start(out_d.ap(), OUTS[:])

    nc.compile()
    return nc


def _weight_map(packs):
    w1b, w2b, fc3t, wiht, whht, fc5t = packs
    return {
        "w1b": w1b, "w2b": w2b,
        "fc3t": fc3t, "wiht": wiht, "whht": whht, "fc5t": fc5t,
    }


# ---------------------------------------------------------------- entry point
def kernel(**inputs):
    x = np.asarray(inputs["x"], np.float32)
    packs = _pack_weights(
        np.asarray(inputs["conv1_w"], np.float32),
        np.asarray(inputs["conv2_w"], np.float32),
        np.asarray(inputs["fc3_w"], np.float32),
        np.asarray(inputs["w_ih"], np.float32),
        np.asarray(inputs["w_hh"], np.float32),
        np.asarray(inputs["fc5_w"], np.float32),
    )
    NCORES = 8
    B = x.shape[0]
    BS = B // NCORES

    relu_ok = _relu_safe(
        x, np.asarray(inputs["conv1_w"], np.float32),
        np.asarray(inputs["conv2_w"], np.float32),
    )
    nc = build_nc(BS=BS, Sc=512, relu_acts=relu_ok)
    in_maps = [dict(_weight_map(packs), xt=_pack_x(x[c * BS : (c + 1) * BS]))
               for c in range(NCORES)]
    res = run_bass_kernel_spmd(nc, in_maps, core_ids=list(range(NCORES)))
    out = np.concatenate([res.results[c]["out"].T for c in range(NCORES)], axis=0)
    return np.ascontiguousarray(out, np.float32)


if __name__ == "__main__":
    rng = np.random.default_rng(0)
    ins = {
        "x": rng.standard_normal((2048, T, HH, WW), np.float32) * 0.5,
        "conv1_w": rng.standard_normal((CH1, 1, 3, 3), np.float32) * 0.1,
        "conv2_w": rng.standard_normal((CH2, CH1, 3, 3), np.float32) * 0.1,
        "fc3_w": rng.standard_normal((256, 3520), np.float32) * 0.1,
        "w_ih": rng.standard_normal((128, 256), np.float32) * 0.1,
        "w_hh": rng.standard_normal((128, HID), np.float32) * 0.1,
        "fc5_w": rng.standard_normal((NCLS, HID), np.float32) * 0.1,
    }
    out = kernel(**ins)
    print(out.shape, out.dtype, np.abs(out).mean())
